# revision 24
# baseline (speedup 1.0000x reference)
"""Chebyshev approximation kernel for Trainium2 (8 NeuronCores, SPMD data-parallel).

Math: reference computes
    y_at_nodes = (1-t) * y[:, idx] + t * y[:, idx+1]      # [n_obs, deg]
    out        = (y_at_nodes @ basis).reshape(-1)         # [n_obs*deg]
Factorized device kernel: out = (y @ W) @ B where W [2049, 1024] holds the
two interp weights per node column and B is the dense basis. W is banded
(idx is monotone), so GEMM1 (ynT = W^T @ y^T) only touches ~26 of the 136
possible [128x128] tile pairs; its output lands in PSUM already transposed
for GEMM2 (contraction 1024 vs 2049 of the fused y@C form). All matmuls in
bf16 (full PE rate, LDWEIGHTS hidden); y is cast bf16 on DVE/ACT before the
PE transposes; GEMM1 runs on m=512 groups (4 row-blocks) to amortize weight
loads. Output stored bf16 (halves store DMA), upcast on host.

Sharding: y rows split 8192/core across 8 cores; W/B replicated. The band
structure (not the W values) is baked at compile time and cached by its
signature, so recompiles only happen if x changes shape qualitatively.
"""

import os
import numpy as np

DEG = 1024
N_OBS = 65536
M_P1 = 2049
N_CORES = 8
ROWS_PER_CORE = N_OBS // N_CORES  # 8192
RB = 128                          # rows per block
GB = 4                            # blocks per GEMM1 group (m = 512)
KT = 17                           # k tiles of 128 covering 2049 (pad to 2176)
KP = KT * 128                     # 2176
JT = 8                            # node j-tiles (1024/128)

_COMPILED = {}
_PREP_CACHE = {}
LAST_RESULTS = None


def _prep(x: np.ndarray):
    """Host precompute: banded W (bf16), basis B (bf16), band structure."""
    import ml_dtypes

    key = x.tobytes()
    hit = _PREP_CACHE.get(key)
    if hit is not None:
        return hit
    x = np.asarray(x, dtype=np.float32)
    k = np.arange(DEG, dtype=np.float32)
    ang = (np.float32(np.pi) * (k + np.float32(0.5))) / np.float32(DEG)
    nodes = np.sort(np.cos(ang.astype(np.float32)).astype(np.float32))
    idx = np.clip(np.searchsorted(x, nodes, side="right") - 1, 0, M_P1 - 2)
    a = x[idx]
    b = x[idx + 1]
    t = ((nodes - a) / (b - a)).astype(np.float64)
    W = np.zeros((KP, DEG), dtype=np.float64)
    W[idx, np.arange(DEG)] += 1.0 - t
    W[idx + 1, np.arange(DEG)] += t

    norm = ((np.float32(2.0) - (k == 0).astype(np.float32)) / np.float32(DEG)).astype(
        np.float64
    )
    theta = np.arccos(nodes.astype(np.float64))
    basis = norm[None, :] * np.cos(k.astype(np.float64)[None, :] * theta[:, None])

    # band: per j-tile, the k-tiles containing any nonzero of W
    bands = []
    for jt in range(JT):
        lo = int(idx[jt * 128 : (jt + 1) * 128].min()) // 128
        hi = int(idx[jt * 128 : (jt + 1) * 128].max() + 1) // 128
        bands.append(tuple(range(lo, hi + 1)))
    bands = tuple(bands)

    # pack W band tiles and B partition-major so each loads as ONE DMA
    # with 128 large contiguous descriptors.
    nband = sum(len(bd) for bd in bands)
    W_pk = np.empty((128, nband * 128), dtype=np.float64)
    s = 0
    for jt, bd in enumerate(bands):
        for kt in bd:
            W_pk[:, s * 128 : (s + 1) * 128] = W[
                kt * 128 : (kt + 1) * 128, jt * 128 : (jt + 1) * 128
            ]
            s += 1
    W_bf = np.ascontiguousarray(W_pk.astype(ml_dtypes.bfloat16))
    B_pk = basis.reshape(JT, 128, DEG).transpose(1, 0, 2).reshape(128, JT * DEG)
    B_bf = np.ascontiguousarray(B_pk.astype(ml_dtypes.bfloat16))
    out = (W_bf, B_bf, bands)
    _PREP_CACHE[key] = out
    return out


def build_cheb_kernel(tc, y_ap, w_ap, b_ap, id_ap, o_ap, rows, bands):
    import concourse.mybir as mybir

    nc = tc.nc
    f32 = mybir.dt.float32
    bf16 = mybir.dt.bfloat16
    nb = rows // RB
    ngrp = nb // GB

    with (
        tc.tile_pool(name="consts", bufs=1) as consts,
        tc.tile_pool(name="ypool", bufs=6) as ypool,
        tc.tile_pool(name="ycpool", bufs=6) as ycpool,
        tc.tile_pool(name="ytg", bufs=2) as ytgpool,
        tc.tile_pool(name="ynt", bufs=2) as yntpool,
        tc.tile_pool(name="opool", bufs=3) as opool,
        tc.tile_pool(name="pst", bufs=3, space="PSUM") as pstp,
        tc.tile_pool(name="p1", bufs=2, space="PSUM") as p1p,
        tc.tile_pool(name="pso", bufs=3, space="PSUM") as psop,
    ):
        ident = consts.tile([128, 128], bf16)
        nc.scalar.dma_start(out=ident, in_=id_ap)
        nband = sum(len(bd) for bd in bands)
        b_sb = consts.tile([128, JT * DEG], bf16)
        w_sb = consts.tile([128, nband * 128], bf16)

        def load_consts():
            # Both are host-packed partition-major: one dma_start each.
            nc.scalar.dma_start(out=w_sb, in_=w_ap)
            nc.scalar.dma_start(out=b_sb, in_=b_ap)
            s = 0
            for jt, bd in enumerate(bands):
                for kt in bd:
                    slot[(jt, kt)] = s
                    s += 1

        slot = {}
        ybs, ycs, ytgs, ynts, psos = {}, {}, {}, {}, {}

        def load_y(b):
            yb = ypool.tile([128, M_P1], f32, name="yb", tag="yb")
            nc.sync.dma_start(out=yb, in_=y_ap[b * RB : (b + 1) * RB, :])
            ybs[b] = yb

        def cast_block(b):
            # gpsimd (otherwise idle) does the f32->bf16 cast so DVE/ACT
            # queues stay clear for PSUM drains.
            yc = ycpool.tile([128, KP], bf16, name="yc", tag="yc")
            nc.gpsimd.memset(yc[:, M_P1:KP], 0.0)
            nc.gpsimd.tensor_copy(yc[:, 0:M_P1], ybs[b])
            ycs[b] = yc
            del ybs[b]

        def trans_block(b):
            g = b % GB
            if g == 0:
                ytgs[b // GB] = ytgpool.tile(
                    [128, KT, GB * 128], bf16, name="ytg", tag="ytg"
                )
            ytg = ytgs[b // GB]
            yc = ycs[b]
            pst = None
            for gg in range(5):  # transpose groups: 4,4,4,4,1
                kts = list(range(gg * 4, min(gg * 4 + 4, KT)))
                # one bank-aligned pst tile serves two groups (subtile-tracked)
                if gg % 2 == 0:
                    pst = pstp.tile([128, 8, 128], bf16, name="pst", tag="pst")
                base = (gg % 2) * 4
                for ji, kt in enumerate(kts):
                    nc.tensor.transpose(
                        pst[:, base + ji, :], yc[:, kt * 128 : (kt + 1) * 128], ident
                    )
                dst = ytg[:, kts[0] : kts[-1] + 1, g * 128 : (g + 1) * 128]
                src = pst[:, base : base + len(kts), :]
                if gg % 2 == 0:
                    nc.vector.tensor_copy(dst, src)
                else:
                    nc.scalar.copy(dst, src)
            del ycs[b]

        def gemm1(grp):
            ytg = ytgs[grp]
            ynt = yntpool.tile([128, JT, GB * 128], bf16, name="ynt", tag="ynt")
            ynts[grp] = ynt
            for jt in range(JT):
                bd = bands[jt]
                p1 = p1p.tile([128, GB * 128], f32, name="p1", tag="p1")
                for i, kt in enumerate(bd):
                    s = slot[(jt, kt)]
                    nc.tensor.matmul(
                        p1,
                        w_sb[:, s * 128 : (s + 1) * 128],
                        ytg[:, kt, :],
                        start=(i == 0),
                        stop=(i == len(bd) - 1),
                    )
                if jt % 2 == 0:
                    nc.vector.tensor_copy(ynt[:, jt, :], p1)
                else:
                    nc.scalar.copy(ynt[:, jt, :], p1)
            del ytgs[grp]

        def gemm2(b):
            g = b % GB
            ynt = ynts[b // GB]
            osb = opool.tile([128, DEG], bf16, name="osb", tag="osb")
            for nh in range(2):
                ps = psop.tile([128, 512], f32, name="ps", tag="ps")
                for jt in range(JT):
                    nc.tensor.matmul(
                        ps,
                        ynt[:, jt, g * 128 : (g + 1) * 128],
                        b_sb[:, jt * DEG + nh * 512 : jt * DEG + (nh + 1) * 512],
                        start=(jt == 0),
                        stop=(jt == JT - 1),
                    )
                if nh == 0:
                    nc.vector.tensor_copy(osb[:, 0:512], ps)
                else:
                    nc.scalar.copy(osb[:, 512:1024], ps)
            nc.scalar.dma_start(out=o_ap[b * RB : (b + 1) * RB, :], in_=osb)
            if g == GB - 1:
                del ynts[b // GB]

        # prologue: first-group y loads beat the constant loads onto the
        # queues; W tiles land before gemm1(0), B before gemm2(0).
        for b in range(min(GB, nb)):
            load_y(b)
        load_consts()
        for b in range(GB, min(2 * GB, nb)):
            load_y(b)
        for b in range(min(GB, nb)):
            cast_block(b)

        for grp in range(ngrp):
            for b in range((grp + 2) * GB, min((grp + 3) * GB, nb)):
                load_y(b)
            for b in range(grp * GB, (grp + 1) * GB):
                trans_block(b)
            for b in range((grp + 1) * GB, min((grp + 2) * GB, nb)):
                cast_block(b)
            gemm1(grp)
            for b in range(grp * GB, (grp + 1) * GB):
                gemm2(b)


def _build_nc(rows, bands):
    import concourse.mybir as mybir
    import concourse.tile as tile
    from concourse import bacc

    f32 = mybir.dt.float32
    bf16 = mybir.dt.bfloat16
    nc = bacc.Bacc(
        "TRN2",
        target_bir_lowering=False,
        debug=False,
        enable_asserts=False,
        num_devices=N_CORES,
    )
    nband = sum(len(bd) for bd in bands)
    y_ap = nc.dram_tensor("y", [rows, M_P1], f32, kind="ExternalInput").ap()
    w_ap = nc.dram_tensor("wmat", [128, nband * 128], bf16, kind="ExternalInput").ap()
    b_ap = nc.dram_tensor("bmat", [128, JT * DEG], bf16, kind="ExternalInput").ap()
    id_ap = nc.dram_tensor("ident", [128, 128], bf16, kind="ExternalInput").ap()
    o_ap = nc.dram_tensor("o", [rows, DEG], bf16, kind="ExternalOutput").ap()
    with tile.TileContext(nc) as tc:
        build_cheb_kernel(tc, y_ap, w_ap, b_ap, id_ap, o_ap, rows, bands)
    nc.compile()
    return nc


def _get_compiled(rows, bands):
    key = (rows, bands)
    if key not in _COMPILED:
        _COMPILED[key] = _build_nc(rows, bands)
    return _COMPILED[key]


def kernel(x: np.ndarray, y: np.ndarray) -> np.ndarray:
    global LAST_RESULTS
    import ml_dtypes
    from concourse import bass_utils

    x = np.asarray(x, dtype=np.float32)
    y = np.ascontiguousarray(np.asarray(y, dtype=np.float32))
    assert y.shape == (N_OBS, M_P1), y.shape
    W_bf, B_bf, bands = _prep(x)

    nc = _get_compiled(ROWS_PER_CORE, bands)
    ident = np.ascontiguousarray(np.eye(128, dtype=ml_dtypes.bfloat16))
    in_maps = [
        {
            "y": y[i * ROWS_PER_CORE : (i + 1) * ROWS_PER_CORE],
            "wmat": W_bf,
            "bmat": B_bf,
            "ident": ident,
        }
        for i in range(N_CORES)
    ]
    trace = bool(int(os.environ.get("CHEB_TRACE", "0")))
    res = bass_utils.run_bass_kernel_spmd(
        nc, in_maps, core_ids=list(range(N_CORES)), trace=trace
    )
    LAST_RESULTS = res
    out = np.concatenate(
        [
            np.asarray(res.results[i]["o"]).astype(np.float32)
            for i in range(N_CORES)
        ],
        axis=0,
    )
    return out.reshape(-1)
